# revision 9
# baseline (speedup 1.0000x reference)
"""Trainium2 Bass kernel for nn_BaseDiscretGenerator (histogram binning).

reference semantics:
    steps = relu(interval_lengths) + 1e-4                    # [D,4]
    b = cumsum([min_boundary, steps])                        # [D,5] strictly increasing
    g_k = (v > b_k)                                          # [B,D,5]
    dist = [1-g0, g0-g1, g1-g2, g2-g3, g3-g4, g4]            # [B,D,6] one-hot bin
    val  = sum_k g_k                                         # [B,D] bin index

Sharded batch-parallel over 8 NeuronCores (256 rows each). Inside each core:
partition dim = batch rows, free dim = feature (D) chunks, all intermediate
tensors in the d-interleaved layout [d*5+k] so each stage is one wide
instruction. Boundaries are computed on-chip, staged to DRAM as an
interleaved flat row, broadcast across the 128 partitions with K=1 PE
matmuls (ones-vector weights, 512-wide bank-aligned), and copied from PSUM
to SBUF by the scalar engine. Per tile: one DVE compare produces all five
g planes, one GPSIMD subtract fills one-hot slots 1-4, the scalar engine
fills slots 0 and 5, and a contiguous-window DVE reduce produces val.
"""

import os
import sys

for _p in ("/opt/trn_rl_repo", "/root/.axon_site", "/root/.axon_site/_ro/trn_rl_repo",
           "/root/.axon_site/_ro/pypackages"):
    if os.path.isdir(_p) and _p not in sys.path:
        sys.path.append(_p)

import numpy as np

from concourse import bass, mybir
from concourse.tile import TileContext
from concourse.bass_utils import run_bass_kernel_spmd

B, D = 2048, 3706
N_CORES = 8
ROWS = B // N_CORES          # 256 rows per core
N_BT = ROWS // 128           # 2 partition tiles per core
EPS = 1e-4
P_B, F_B = 109, 34           # D = 109 * 34 for the boundary-prep layout
CHUNKS = [371] * 6 + [370] * 4  # sums to 3706

F32 = mybir.dt.float32
AF = mybir.ActivationFunctionType
ALU = mybir.AluOpType


def _split_excess_waits(nc, max_waits=1):
    """The walrus build in this environment rejects instructions carrying
    more than one semaphore wait. Move excess waits onto preceding
    same-engine NOPs (same-engine program order preserves semantics)."""
    n = 0
    for fn in nc.m.functions:
        for bb in fn.blocks:
            insts = list(bb.instructions)
            if not any(i.sync_info is not None and len(i.sync_info.on_wait) > max_waits
                       for i in insts):
                continue
            new_insts = []
            for ins in insts:
                si = ins.sync_info
                if si is not None and len(si.on_wait) > max_waits:
                    waits = list(si.on_wait)
                    extra, keep = waits[:-max_waits], waits[-max_waits:]
                    idx = 0
                    while extra:
                        chunk, extra = extra[:max_waits], extra[max_waits:]
                        nop = mybir.InstNoOp(
                            name=f"{ins.name}-waitsplit{idx}",
                            sync_info=mybir.SyncInfo(on_wait=chunk, on_update=[]),
                            engine=ins.engine,
                            bass_nofuse=True,
                        )
                        nc.register_instruction(nop, overwrite=True)
                        new_insts.append(nop)
                        idx += 1
                        n += 1
                    ins.sync_info = mybir.SyncInfo(on_wait=keep,
                                                   on_update=list(si.on_update))
                new_insts.append(ins)
            bb.instructions = new_insts
    return n


def _build_nc():
    nc = bass.Bass()
    fake = nc.dram_tensor("fake", [ROWS, D], F32, kind="ExternalInput")
    minb = nc.dram_tensor("minb", [D], F32, kind="ExternalInput")
    ilen = nc.dram_tensor("ilen", [D, 4], F32, kind="ExternalInput")
    dist = nc.dram_tensor("dist", [ROWS, D, 6], F32, kind="ExternalOutput")
    val = nc.dram_tensor("val", [ROWS, D], F32, kind="ExternalOutput")

    with TileContext(nc) as tc:
        with (
            tc.tile_pool(name="dram", bufs=1, space="DRAM") as drampool,
            tc.tile_pool(name="setup", bufs=1) as setup_pool,
            tc.tile_pool(name="vin", bufs=2) as vpool,
            tc.tile_pool(name="brow", bufs=2) as browpool,
            tc.tile_pool(name="bsb", bufs=2) as bpool,
            tc.tile_pool(name="g", bufs=2) as gpool,
            tc.tile_pool(name="dist", bufs=3) as dpool,
            tc.tile_pool(name="valp", bufs=2) as valpool,
            tc.tile_pool(name="psum", bufs=2, space="PSUM") as ppool,
        ):
            # boundary staging row in DRAM (pool tile => RAW deps tracked),
            # flat d-interleaved: b_flat[d*5 + k] = b[d, k]
            b_dram_t = drampool.tile([1, 5 * D], F32)
            b_dram = b_dram_t[0, :]

            # ---- boundary prep: b[d,k] on a [109, 5, 34] grid (d = p*34+f) ----
            t_min = setup_pool.tile([P_B, F_B], F32)
            nc.sync.dma_start(out=t_min[:, :],
                              in_=minb[:].rearrange("(p f) -> p f", p=P_B))
            t_len = setup_pool.tile([P_B, F_B * 4], F32)
            nc.sync.dma_start(
                out=t_len[:, :].rearrange("p (f k) -> p f k", k=4),
                in_=ilen[:, :].rearrange("(p f) k -> p f k", p=P_B))
            # steps = relu(ilen) + eps, layout [109, (f,k)]
            nc.scalar.activation(t_len[:, :], t_len[:, :], AF.Relu)
            nc.vector.tensor_scalar_add(t_len[:, :], t_len[:, :], EPS)
            # t_b interleaved per partition: t_b[p, f*5 + k] = b[p*34+f, k]
            t_b = setup_pool.tile([P_B, 5 * F_B], F32)
            t_b_v = t_b[:, :].rearrange("p (f k) -> p f k", k=5)
            steps_v = t_len[:, :].rearrange("p (f k) -> p f k", k=4)
            nc.vector.tensor_copy(t_b_v[:, :, 0], t_min[:, :])
            for k in range(1, 5):
                nc.vector.tensor_add(
                    out=t_b_v[:, :, k],
                    in0=t_b_v[:, :, k - 1],
                    in1=steps_v[:, :, k - 1])
            # stage to DRAM: b_flat[(p*34+f)*5 + k] -- contiguous per partition
            nc.sync.dma_start(
                out=b_dram[:].rearrange("(p x) -> p x", p=P_B),
                in_=t_b[:, :])

            # ones weights for the K=1 broadcast matmul
            ones = setup_pool.tile([1, 128], F32)
            nc.vector.memset(ones[:, :], 1.0)

            # ---- main loop ----
            v_t = [vpool.tile([128, D], F32, tag="v", name=f"v{bt}")
                   for bt in range(N_BT)]
            for bt in range(N_BT):
                nc.sync.dma_start(out=v_t[bt][:, :],
                                  in_=fake[bt * 128:(bt + 1) * 128, :])
            val_t = [valpool.tile([128, D], F32, tag="val", name=f"val{bt}")
                     for bt in range(N_BT)]

            d0 = 0
            for w in CHUNKS:
                n = 5 * w
                # contiguous interleaved slice of the staged boundaries
                b_row = browpool.tile([1, 5 * 512], F32, tag="brow")
                nc.sync.dma_start(
                    out=b_row[0:1, :n],
                    in_=b_dram[5 * d0:5 * d0 + n].rearrange("(q x) -> q x", q=1))
                # replicate across partitions: 4 bank-aligned 512-wide matmuls
                pt = ppool.tile([128, 2048], F32, tag="pt")
                for m0 in range(0, n, 512):
                    mw = min(512, n - m0)
                    nc.tensor.matmul(pt[:, m0:m0 + mw], ones[0:1, :],
                                     b_row[0:1, m0:m0 + mw],
                                     start=True, stop=True)
                B_sb = bpool.tile([128, 5 * 512], F32, tag="bsb")
                nc.scalar.copy(B_sb[:, :n], pt[:, :n])
                for bt in range(N_BT):
                    vv = v_t[bt][:, d0:d0 + w]
                    g = gpool.tile([128, 5 * 512], F32, tag="g")
                    g_v = g[:, :n].rearrange("p (d k) -> p d k", k=5)
                    # one compare: g[p, d, k] = v[p, d] > B[d, k]
                    nc.vector.tensor_tensor(
                        out=g_v, in0=vv.broadcast_to([128, w, 5]),
                        in1=B_sb[:, :n].rearrange("p (d k) -> p d k", k=5),
                        op=ALU.is_gt)
                    dist_t = dpool.tile([128, 512 * 6], F32, tag="dist")
                    dist_v = dist_t[:, :w * 6].rearrange("p (d r) -> p d r", r=6)
                    # slots 1-4 in one strided subtract on the (idle) pool engine
                    nc.gpsimd.tensor_tensor(
                        out=dist_v[:, :, 1:5],
                        in0=g_v[:, :, 0:4], in1=g_v[:, :, 1:5], op=ALU.subtract)
                    nc.scalar.activation(dist_v[:, :, 0], g_v[:, :, 0], AF.Copy,
                                         bias=1.0, scale=-1.0)
                    nc.scalar.copy(dist_v[:, :, 5], g_v[:, :, 4])
                    nc.vector.tensor_reduce(
                        out=val_t[bt][:, d0:d0 + w], in_=g_v,
                        axis=mybir.AxisListType.X, op=ALU.add)
                    nc.sync.dma_start(
                        out=dist[bt * 128:(bt + 1) * 128, d0:d0 + w, :],
                        in_=dist_v[:, :, :])
                d0 += w

            for bt in range(N_BT):
                nc.sync.dma_start(out=val[bt * 128:(bt + 1) * 128, :],
                                  in_=val_t[bt][:, :])

    _split_excess_waits(nc, 1)
    return nc


_NC_CACHE = {}


def _get_nc():
    if "nc" not in _NC_CACHE:
        _NC_CACHE["nc"] = _build_nc()
    return _NC_CACHE["nc"]


def _patch_exact_ties(dist, val, fake, minb, ilen):
    """The device kernel computes dist_r = gt_{r-1} - gt_r, which differs from
    the reference H-product only where fake == b_k EXACTLY (the reference
    yields an all-zero one-hot row and val 0 there; heaviside H(0)=0 on both
    sides of the boundary). Ties are measure-zero; recompute the reference
    formula at just those sites."""
    steps = np.maximum(ilen, 0.0).astype(np.float32) + np.float32(EPS)
    b = np.cumsum(np.concatenate([minb[:, None], steps], axis=1).astype(np.float32),
                  axis=1, dtype=np.float32)  # [D,5]
    rows = np.zeros((0,), np.int64)
    cols = np.zeros((0,), np.int64)
    for k in range(5):
        r, c = np.nonzero(fake == b[None, :, k])
        rows = np.concatenate([rows, r])
        cols = np.concatenate([cols, c])
    if len(rows) == 0:
        return
    for i, d in zip(rows, cols):
        v = fake[i, d]
        gt = (v > b[d]).astype(np.float32)   # [5]
        lt = (v < b[d]).astype(np.float32)
        row = np.empty(6, np.float32)
        row[0] = lt[0]
        for r in range(1, 5):
            row[r] = gt[r - 1] * lt[r]
        row[5] = gt[4]
        dist[i, d, :] = row
        val[i, d] = np.dot(row, np.arange(6.0, dtype=np.float32))


def kernel(fake_tensor, min_boundary_value, interval_lengths):
    fake_tensor = np.ascontiguousarray(np.asarray(fake_tensor, dtype=np.float32))
    minb = np.ascontiguousarray(np.asarray(min_boundary_value, dtype=np.float32))
    ilen = np.ascontiguousarray(np.asarray(interval_lengths, dtype=np.float32))
    assert fake_tensor.shape == (B, D)

    nc = _get_nc()
    in_maps = []
    for c in range(N_CORES):
        in_maps.append({
            "fake": fake_tensor[c * ROWS:(c + 1) * ROWS],
            "minb": minb,
            "ilen": ilen,
        })
    res = run_bass_kernel_spmd(nc, in_maps, core_ids=list(range(N_CORES)))
    dist = np.concatenate([res.results[c]["dist"] for c in range(N_CORES)], axis=0)
    val = np.concatenate([res.results[c]["val"] for c in range(N_CORES)], axis=0)
    _patch_exact_ties(dist, val, fake_tensor, minb, ilen)
    return dist, val


# revision 11
# speedup vs baseline: 1.0038x; 1.0038x over previous
"""Trainium2 Bass kernel for nn_BaseDiscretGenerator (histogram binning).

reference semantics:
    steps = relu(interval_lengths) + 1e-4                    # [D,4]
    b = cumsum([min_boundary, steps])                        # [D,5] strictly increasing
    g_k = (v > b_k)                                          # [B,D,5]
    dist = [1-g0, g0-g1, g1-g2, g2-g3, g3-g4, g4]            # [B,D,6] one-hot bin
    val  = sum_k g_k                                         # [B,D] bin index

Sharded batch-parallel over 8 NeuronCores (256 rows each). Inside each core:
partition dim = batch rows, free dim = feature (D) chunks, all intermediate
tensors in the d-interleaved layout [d*5+k] so each stage is one wide
instruction. Boundaries are computed on-chip, staged to DRAM as an
interleaved flat row, broadcast across the 128 partitions with K=1 PE
matmuls (ones-vector weights, 512-wide bank-aligned), and copied from PSUM
to SBUF by the scalar engine. Per tile: one DVE compare produces all five
g planes, one GPSIMD subtract fills one-hot slots 1-4, the scalar engine
fills slots 0 and 5, and a contiguous-window DVE reduce produces val.
"""

import os
import sys

for _p in ("/opt/trn_rl_repo", "/root/.axon_site", "/root/.axon_site/_ro/trn_rl_repo",
           "/root/.axon_site/_ro/pypackages"):
    if os.path.isdir(_p) and _p not in sys.path:
        sys.path.append(_p)

import numpy as np

from concourse import bass, mybir
from concourse.tile import TileContext
from concourse.bass_utils import run_bass_kernel_spmd

B, D = 2048, 3706
N_CORES = 8
ROWS = B // N_CORES          # 256 rows per core
N_BT = ROWS // 128           # 2 partition tiles per core
EPS = 1e-4
P_B, F_B = 109, 34           # D = 109 * 34 for the boundary-prep layout
CHUNKS = [371] * 6 + [370] * 4  # sums to 3706

F32 = mybir.dt.float32
AF = mybir.ActivationFunctionType
ALU = mybir.AluOpType


def _split_excess_waits(nc, max_waits=1):
    """The walrus build in this environment rejects instructions carrying
    more than one semaphore wait. Move excess waits onto preceding
    same-engine NOPs (same-engine program order preserves semantics)."""
    n = 0
    for fn in nc.m.functions:
        for bb in fn.blocks:
            insts = list(bb.instructions)
            if not any(i.sync_info is not None and len(i.sync_info.on_wait) > max_waits
                       for i in insts):
                continue
            new_insts = []
            for ins in insts:
                si = ins.sync_info
                if si is not None and len(si.on_wait) > max_waits:
                    waits = list(si.on_wait)
                    extra, keep = waits[:-max_waits], waits[-max_waits:]
                    idx = 0
                    while extra:
                        chunk, extra = extra[:max_waits], extra[max_waits:]
                        nop = mybir.InstNoOp(
                            name=f"{ins.name}-waitsplit{idx}",
                            sync_info=mybir.SyncInfo(on_wait=chunk, on_update=[]),
                            engine=ins.engine,
                            bass_nofuse=True,
                        )
                        nc.register_instruction(nop, overwrite=True)
                        new_insts.append(nop)
                        idx += 1
                        n += 1
                    ins.sync_info = mybir.SyncInfo(on_wait=keep,
                                                   on_update=list(si.on_update))
                new_insts.append(ins)
            bb.instructions = new_insts
    return n


def _build_nc():
    nc = bass.Bass()
    fake = nc.dram_tensor("fake", [ROWS, D], F32, kind="ExternalInput")
    minb = nc.dram_tensor("minb", [D], F32, kind="ExternalInput")
    ilen = nc.dram_tensor("ilen", [D, 4], F32, kind="ExternalInput")
    dist = nc.dram_tensor("dist", [ROWS, D, 6], F32, kind="ExternalOutput")
    val = nc.dram_tensor("val", [ROWS, D], F32, kind="ExternalOutput")

    with TileContext(nc) as tc:
        with (
            tc.tile_pool(name="dram", bufs=1, space="DRAM") as drampool,
            tc.tile_pool(name="setup", bufs=1) as setup_pool,
            tc.tile_pool(name="vin", bufs=2) as vpool,
            tc.tile_pool(name="brow", bufs=3) as browpool,
            tc.tile_pool(name="g", bufs=3) as gpool,
            tc.tile_pool(name="dist", bufs=4) as dpool,
            tc.tile_pool(name="valp", bufs=2) as valpool,
            tc.tile_pool(name="psum", bufs=2, space="PSUM") as ppool,
        ):
            # boundary staging row in DRAM (pool tile => RAW deps tracked),
            # flat d-interleaved: b_flat[d*5 + k] = b[d, k]
            b_dram_t = drampool.tile([1, 5 * D], F32)
            b_dram = b_dram_t[0, :]

            # input rows first so their DMA overlaps the boundary prep
            v_t = [vpool.tile([128, D], F32, tag="v", name=f"v{bt}")
                   for bt in range(N_BT)]
            for bt in range(N_BT):
                nc.sync.dma_start(out=v_t[bt][:, :],
                                  in_=fake[bt * 128:(bt + 1) * 128, :])

            # ---- boundary prep: b[d,k] on a [109, 5, 34] grid (d = p*34+f) ----
            t_min = setup_pool.tile([P_B, F_B], F32)
            nc.sync.dma_start(out=t_min[:, :],
                              in_=minb[:].rearrange("(p f) -> p f", p=P_B))
            t_len = setup_pool.tile([P_B, F_B * 4], F32)
            nc.sync.dma_start(
                out=t_len[:, :].rearrange("p (f k) -> p f k", k=4),
                in_=ilen[:, :].rearrange("(p f) k -> p f k", p=P_B))
            # steps = relu(ilen) + eps, layout [109, (f,k)]
            nc.scalar.activation(t_len[:, :], t_len[:, :], AF.Relu)
            nc.vector.tensor_scalar_add(t_len[:, :], t_len[:, :], EPS)
            # t_b interleaved per partition: t_b[p, f*5 + k] = b[p*34+f, k]
            t_b = setup_pool.tile([P_B, 5 * F_B], F32)
            t_b_v = t_b[:, :].rearrange("p (f k) -> p f k", k=5)
            steps_v = t_len[:, :].rearrange("p (f k) -> p f k", k=4)
            nc.vector.tensor_copy(t_b_v[:, :, 0], t_min[:, :])
            for k in range(1, 5):
                nc.vector.tensor_add(
                    out=t_b_v[:, :, k],
                    in0=t_b_v[:, :, k - 1],
                    in1=steps_v[:, :, k - 1])
            # stage to DRAM: b_flat[(p*34+f)*5 + k] -- contiguous per partition
            nc.sync.dma_start(
                out=b_dram[:].rearrange("(p x) -> p x", p=P_B),
                in_=t_b[:, :])

            # ones weights for the K=1 broadcast matmul
            ones = setup_pool.tile([1, 128], F32)
            nc.vector.memset(ones[:, :], 1.0)

            # ---- main loop ----
            val_t = [valpool.tile([128, D], F32, tag="val", name=f"val{bt}")
                     for bt in range(N_BT)]

            NMAX = 5 * max(CHUNKS)
            d0 = 0
            val_flushed = 0
            for ci, w in enumerate(CHUNKS):
                n = 5 * w
                # contiguous interleaved slice of the staged boundaries
                b_row = browpool.tile([1, NMAX], F32, tag="brow")
                nc.sync.dma_start(
                    out=b_row[0:1, :n],
                    in_=b_dram[5 * d0:5 * d0 + n].rearrange("(q x) -> q x", q=1))
                # replicate across partitions: 4 bank-aligned 512-wide matmuls;
                # compares read the result straight from PSUM
                pt = ppool.tile([128, 2048], F32, tag="pt")
                for m0 in range(0, n, 512):
                    mw = min(512, n - m0)
                    nc.tensor.matmul(pt[:, m0:m0 + mw], ones[0:1, :],
                                     b_row[0:1, m0:m0 + mw],
                                     start=True, stop=True)
                B_v = pt[:, :n].rearrange("p (d k) -> p d k", k=5)
                for bt in range(N_BT):
                    vv = v_t[bt][:, d0:d0 + w]
                    g = gpool.tile([128, NMAX], mybir.dt.bfloat16, tag="g")
                    g_v = g[:, :n].rearrange("p (d k) -> p d k", k=5)
                    # one compare: g[p, d, k] = v[p, d] > B[d, k]
                    nc.vector.tensor_tensor(
                        out=g_v, in0=vv.broadcast_to([128, w, 5]),
                        in1=B_v, op=ALU.is_gt)
                    dist_t = dpool.tile([128, max(CHUNKS) * 6], F32, tag="dist")
                    dist_v = dist_t[:, :w * 6].rearrange("p (d r) -> p d r", r=6)
                    # slots 1-4 in one strided subtract on the (idle) pool engine
                    nc.gpsimd.tensor_tensor(
                        out=dist_v[:, :, 1:5],
                        in0=g_v[:, :, 0:4], in1=g_v[:, :, 1:5], op=ALU.subtract)
                    nc.scalar.activation(dist_v[:, :, 0], g_v[:, :, 0], AF.Copy,
                                         bias=1.0, scale=-1.0)
                    nc.scalar.copy(dist_v[:, :, 5], g_v[:, :, 4])
                    nc.vector.tensor_reduce(
                        out=val_t[bt][:, d0:d0 + w], in_=g_v,
                        axis=mybir.AxisListType.X, op=ALU.add)
                    nc.sync.dma_start(
                        out=dist[bt * 128:(bt + 1) * 128, d0:d0 + w, :],
                        in_=dist_v[:, :, :])
                d0 += w
                # flush val halves early so the final stores don't serialize
                if ci == len(CHUNKS) // 2 - 1 or ci == len(CHUNKS) - 1:
                    for bt in range(N_BT):
                        nc.sync.dma_start(
                            out=val[bt * 128:(bt + 1) * 128, val_flushed:d0],
                            in_=val_t[bt][:, val_flushed:d0])
                    val_flushed = d0

    _split_excess_waits(nc, 1)
    return nc


_NC_CACHE = {}


def _get_nc():
    if "nc" not in _NC_CACHE:
        _NC_CACHE["nc"] = _build_nc()
    return _NC_CACHE["nc"]


def _patch_exact_ties(dist, val, fake, minb, ilen):
    """The device kernel computes dist_r = gt_{r-1} - gt_r, which differs from
    the reference H-product only where fake == b_k EXACTLY (the reference
    yields an all-zero one-hot row and val 0 there; heaviside H(0)=0 on both
    sides of the boundary). Ties are measure-zero; recompute the reference
    formula at just those sites."""
    steps = np.maximum(ilen, 0.0).astype(np.float32) + np.float32(EPS)
    b = np.cumsum(np.concatenate([minb[:, None], steps], axis=1).astype(np.float32),
                  axis=1, dtype=np.float32)  # [D,5]
    rows = np.zeros((0,), np.int64)
    cols = np.zeros((0,), np.int64)
    for k in range(5):
        r, c = np.nonzero(fake == b[None, :, k])
        rows = np.concatenate([rows, r])
        cols = np.concatenate([cols, c])
    if len(rows) == 0:
        return
    for i, d in zip(rows, cols):
        v = fake[i, d]
        gt = (v > b[d]).astype(np.float32)   # [5]
        lt = (v < b[d]).astype(np.float32)
        row = np.empty(6, np.float32)
        row[0] = lt[0]
        for r in range(1, 5):
            row[r] = gt[r - 1] * lt[r]
        row[5] = gt[4]
        dist[i, d, :] = row
        val[i, d] = np.dot(row, np.arange(6.0, dtype=np.float32))


def kernel(fake_tensor, min_boundary_value, interval_lengths):
    fake_tensor = np.ascontiguousarray(np.asarray(fake_tensor, dtype=np.float32))
    minb = np.ascontiguousarray(np.asarray(min_boundary_value, dtype=np.float32))
    ilen = np.ascontiguousarray(np.asarray(interval_lengths, dtype=np.float32))
    assert fake_tensor.shape == (B, D)

    nc = _get_nc()
    in_maps = []
    for c in range(N_CORES):
        in_maps.append({
            "fake": fake_tensor[c * ROWS:(c + 1) * ROWS],
            "minb": minb,
            "ilen": ilen,
        })
    res = run_bass_kernel_spmd(nc, in_maps, core_ids=list(range(N_CORES)))
    dist = np.concatenate([res.results[c]["dist"] for c in range(N_CORES)], axis=0)
    val = np.concatenate([res.results[c]["val"] for c in range(N_CORES)], axis=0)
    _patch_exact_ties(dist, val, fake_tensor, minb, ilen)
    return dist, val


# revision 16
# speedup vs baseline: 1.0848x; 1.0806x over previous
"""Trainium2 Bass kernel for nn_BaseDiscretGenerator (histogram binning).

reference semantics:
    steps = relu(interval_lengths) + 1e-4                    # [D,4]
    b = cumsum([min_boundary, steps])                        # [D,5] strictly increasing
    g_k = (v > b_k)                                          # [B,D,5]
    dist = [1-g0, g0-g1, g1-g2, g2-g3, g3-g4, g4]            # [B,D,6] one-hot bin
    val  = sum_k g_k                                         # [B,D] bin index

Sharded batch-parallel over 8 NeuronCores (256 rows each). Inside each core:
partition dim = batch rows, free dim = feature (D) chunks, all intermediate
tensors in the d-interleaved layout [d*5+k] so each stage is one wide
instruction. Boundaries are computed on-chip, staged to DRAM as an
interleaved flat row, broadcast across the 128 partitions with K=1 PE
matmuls (ones-vector weights, 512-wide bank-aligned), and copied from PSUM
to SBUF by the scalar engine. Per tile: one DVE compare produces all five
g planes, one GPSIMD subtract fills one-hot slots 1-4, the scalar engine
fills slots 0 and 5, and a contiguous-window DVE reduce produces val.
"""

import os
import sys

for _p in ("/opt/trn_rl_repo", "/root/.axon_site", "/root/.axon_site/_ro/trn_rl_repo",
           "/root/.axon_site/_ro/pypackages"):
    if os.path.isdir(_p) and _p not in sys.path:
        sys.path.append(_p)

import numpy as np

from concourse import bass, mybir
from concourse.tile import TileContext
from concourse.bass_utils import run_bass_kernel_spmd

B, D = 2048, 3706
N_CORES = 8
ROWS = B // N_CORES          # 256 rows per core
N_BT = ROWS // 128           # 2 partition tiles per core
EPS = 1e-4
P_B, F_B = 109, 34           # D = 109 * 34 for the boundary-prep layout
# partition-aligned chunks: each spans whole t_b partitions (10*34 / 9*34)
CHUNKS = [340] * 10 + [306]  # sums to 3706

F32 = mybir.dt.float32
AF = mybir.ActivationFunctionType
ALU = mybir.AluOpType


def _split_excess_waits(nc, max_waits=1):
    """The walrus build in this environment rejects instructions carrying
    more than one semaphore wait. Move excess waits onto preceding
    same-engine NOPs (same-engine program order preserves semantics)."""
    n = 0
    for fn in nc.m.functions:
        for bb in fn.blocks:
            insts = list(bb.instructions)
            if not any(i.sync_info is not None and len(i.sync_info.on_wait) > max_waits
                       for i in insts):
                continue
            new_insts = []
            for ins in insts:
                si = ins.sync_info
                if si is not None and len(si.on_wait) > max_waits:
                    waits = list(si.on_wait)
                    extra, keep = waits[:-max_waits], waits[-max_waits:]
                    idx = 0
                    while extra:
                        chunk, extra = extra[:max_waits], extra[max_waits:]
                        nop = mybir.InstNoOp(
                            name=f"{ins.name}-waitsplit{idx}",
                            sync_info=mybir.SyncInfo(on_wait=chunk, on_update=[]),
                            engine=ins.engine,
                            bass_nofuse=True,
                        )
                        nc.register_instruction(nop, overwrite=True)
                        new_insts.append(nop)
                        idx += 1
                        n += 1
                    ins.sync_info = mybir.SyncInfo(on_wait=keep,
                                                   on_update=list(si.on_update))
                new_insts.append(ins)
            bb.instructions = new_insts
    return n


def _build_nc():
    nc = bass.Bass()
    fake = nc.dram_tensor("fake", [ROWS, D], F32, kind="ExternalInput")
    minb = nc.dram_tensor("minb", [D], F32, kind="ExternalInput")
    ilen = nc.dram_tensor("ilen", [D, 4], F32, kind="ExternalInput")
    dist = nc.dram_tensor("dist", [ROWS, D, 6], F32, kind="ExternalOutput")
    val = nc.dram_tensor("val", [ROWS, D], F32, kind="ExternalOutput")

    with TileContext(nc) as tc:
        with (
            tc.tile_pool(name="setup", bufs=1) as setup_pool,
            tc.tile_pool(name="vin", bufs=2) as vpool,
            tc.tile_pool(name="brow", bufs=3) as browpool,
            tc.tile_pool(name="g", bufs=3) as gpool,
            tc.tile_pool(name="dist", bufs=4) as dpool,
            tc.tile_pool(name="valp", bufs=2) as valpool,
            tc.tile_pool(name="psum", bufs=2, space="PSUM") as ppool,
        ):
            # input rows first so their DMA overlaps the boundary prep
            v_t = [vpool.tile([128, D], F32, tag="v", name=f"v{bt}")
                   for bt in range(N_BT)]
            for bt in range(N_BT):
                nc.sync.dma_start(out=v_t[bt][:, :],
                                  in_=fake[bt * 128:(bt + 1) * 128, :])

            # ---- boundary prep: b[d,k] on a [109, 5, 34] grid (d = p*34+f) ----
            t_min = setup_pool.tile([P_B, F_B], F32)
            nc.sync.dma_start(out=t_min[:, :],
                              in_=minb[:].rearrange("(p f) -> p f", p=P_B))
            t_len = setup_pool.tile([P_B, F_B * 4], F32)
            nc.sync.dma_start(
                out=t_len[:, :].rearrange("p (f k) -> p f k", k=4),
                in_=ilen[:, :].rearrange("(p f) k -> p f k", p=P_B))
            # steps = relu(ilen) + eps, layout [109, (f,k)]
            nc.scalar.activation(t_len[:, :], t_len[:, :], AF.Relu)
            nc.vector.tensor_scalar_add(t_len[:, :], t_len[:, :], EPS)
            # t_b interleaved per partition: t_b[p, f*5 + k] = b[p*34+f, k]
            t_b = setup_pool.tile([P_B, 5 * F_B], F32)
            t_b_v = t_b[:, :].rearrange("p (f k) -> p f k", k=5)
            steps_v = t_len[:, :].rearrange("p (f k) -> p f k", k=4)
            nc.vector.tensor_copy(t_b_v[:, :, 0], t_min[:, :])
            for k in range(1, 5):
                nc.vector.tensor_add(
                    out=t_b_v[:, :, k],
                    in0=t_b_v[:, :, k - 1],
                    in1=steps_v[:, :, k - 1])
            # chunks are whole-partition spans of t_b, so each chunk's
            # boundary row is one SBUF->SBUF DMA (no DRAM staging)

            # ones weights for the K=1 broadcast matmul
            ones = setup_pool.tile([1, 128], F32)
            nc.vector.memset(ones[:, :], 1.0)

            # ---- main loop ----
            val_t = [valpool.tile([128, D], F32, tag="val", name=f"val{bt}")
                     for bt in range(N_BT)]

            NMAX = 5 * max(CHUNKS)
            d0 = 0
            val_flushed = 0
            for ci, w in enumerate(CHUNKS):
                n = 5 * w
                np_chunk = w // F_B          # whole t_b partitions in this chunk
                p0 = d0 // F_B
                b_row = browpool.tile([1, NMAX], F32, tag="brow")
                nc.sync.dma_start(
                    out=b_row[0:1, :n].rearrange("q (p x) -> q p x", p=np_chunk),
                    in_=t_b[p0:p0 + np_chunk, :])
                # replicate across partitions: 4 bank-aligned 512-wide matmuls;
                # compares read the result straight from PSUM
                pt = ppool.tile([128, 2048], F32, tag="pt")
                for m0 in range(0, n, 512):
                    mw = min(512, n - m0)
                    nc.tensor.matmul(pt[:, m0:m0 + mw], ones[0:1, :],
                                     b_row[0:1, m0:m0 + mw],
                                     start=True, stop=True)
                B_v = pt[:, :n].rearrange("p (d k) -> p d k", k=5)
                for bt in range(N_BT):
                    vv = v_t[bt][:, d0:d0 + w]
                    g = gpool.tile([128, NMAX], mybir.dt.bfloat16, tag="g")
                    g_v = g[:, :n].rearrange("p (d k) -> p d k", k=5)
                    # one compare: g[p, d, k] = v[p, d] > B[d, k]
                    nc.vector.tensor_tensor(
                        out=g_v, in0=vv.broadcast_to([128, w, 5]),
                        in1=B_v, op=ALU.is_gt)
                    dist_t = dpool.tile([128, max(CHUNKS) * 6], F32, tag="dist")
                    dist_v = dist_t[:, :w * 6].rearrange("p (d r) -> p d r", r=6)
                    # slots 1-4 in one strided subtract on the (idle) pool engine
                    nc.gpsimd.tensor_tensor(
                        out=dist_v[:, :, 1:5],
                        in0=g_v[:, :, 0:4], in1=g_v[:, :, 1:5], op=ALU.subtract)
                    nc.scalar.activation(dist_v[:, :, 0], g_v[:, :, 0], AF.Copy,
                                         bias=1.0, scale=-1.0)
                    nc.scalar.copy(dist_v[:, :, 5], g_v[:, :, 4])
                    nc.vector.tensor_reduce(
                        out=val_t[bt][:, d0:d0 + w], in_=g_v,
                        axis=mybir.AxisListType.X, op=ALU.add)
                    nc.sync.dma_start(
                        out=dist[bt * 128:(bt + 1) * 128, d0:d0 + w, :],
                        in_=dist_v[:, :, :])
                d0 += w
                # flush val halves early so the final stores don't serialize
                if ci == len(CHUNKS) // 2 - 1 or ci == len(CHUNKS) - 1:
                    for bt in range(N_BT):
                        nc.sync.dma_start(
                            out=val[bt * 128:(bt + 1) * 128, val_flushed:d0],
                            in_=val_t[bt][:, val_flushed:d0])
                    val_flushed = d0

    _split_excess_waits(nc, 1)
    return nc


_NC_CACHE = {}


def _get_nc():
    if "nc" not in _NC_CACHE:
        _NC_CACHE["nc"] = _build_nc()
    return _NC_CACHE["nc"]


def _patch_exact_ties(dist, val, fake, minb, ilen):
    """The device kernel computes dist_r = gt_{r-1} - gt_r, which differs from
    the reference H-product only where fake == b_k EXACTLY (the reference
    yields an all-zero one-hot row and val 0 there; heaviside H(0)=0 on both
    sides of the boundary). Ties are measure-zero; recompute the reference
    formula at just those sites."""
    steps = np.maximum(ilen, 0.0).astype(np.float32) + np.float32(EPS)
    b = np.cumsum(np.concatenate([minb[:, None], steps], axis=1).astype(np.float32),
                  axis=1, dtype=np.float32)  # [D,5]
    rows = np.zeros((0,), np.int64)
    cols = np.zeros((0,), np.int64)
    for k in range(5):
        r, c = np.nonzero(fake == b[None, :, k])
        rows = np.concatenate([rows, r])
        cols = np.concatenate([cols, c])
    if len(rows) == 0:
        return
    for i, d in zip(rows, cols):
        v = fake[i, d]
        gt = (v > b[d]).astype(np.float32)   # [5]
        lt = (v < b[d]).astype(np.float32)
        row = np.empty(6, np.float32)
        row[0] = lt[0]
        for r in range(1, 5):
            row[r] = gt[r - 1] * lt[r]
        row[5] = gt[4]
        dist[i, d, :] = row
        val[i, d] = np.dot(row, np.arange(6.0, dtype=np.float32))


def kernel(fake_tensor, min_boundary_value, interval_lengths):
    fake_tensor = np.ascontiguousarray(np.asarray(fake_tensor, dtype=np.float32))
    minb = np.ascontiguousarray(np.asarray(min_boundary_value, dtype=np.float32))
    ilen = np.ascontiguousarray(np.asarray(interval_lengths, dtype=np.float32))
    assert fake_tensor.shape == (B, D)

    nc = _get_nc()
    in_maps = []
    for c in range(N_CORES):
        in_maps.append({
            "fake": fake_tensor[c * ROWS:(c + 1) * ROWS],
            "minb": minb,
            "ilen": ilen,
        })
    res = run_bass_kernel_spmd(nc, in_maps, core_ids=list(range(N_CORES)))
    dist = np.concatenate([res.results[c]["dist"] for c in range(N_CORES)], axis=0)
    val = np.concatenate([res.results[c]["val"] for c in range(N_CORES)], axis=0)
    _patch_exact_ties(dist, val, fake_tensor, minb, ilen)
    return dist, val


# revision 23
# speedup vs baseline: 1.1019x; 1.0157x over previous
"""Trainium2 Bass kernel for nn_BaseDiscretGenerator (histogram binning).

reference semantics:
    steps = relu(interval_lengths) + 1e-4                    # [D,4]
    b = cumsum([min_boundary, steps])                        # [D,5] strictly increasing
    g_k = (v > b_k)                                          # [B,D,5]
    dist = [1-g0, g0-g1, g1-g2, g2-g3, g3-g4, g4]            # [B,D,6] one-hot bin
    val  = sum_k g_k                                         # [B,D] bin index

Sharded batch-parallel over 8 NeuronCores (256 rows each). Inside each core:
partition dim = batch rows, free dim = feature (D) chunks, all intermediate
tensors in the d-interleaved layout [d*5+k] so each stage is one wide
instruction. Boundaries are computed on-chip, staged to DRAM as an
interleaved flat row, broadcast across the 128 partitions with K=1 PE
matmuls (ones-vector weights, 512-wide bank-aligned), and copied from PSUM
to SBUF by the scalar engine. Per tile: one DVE compare produces all five
g planes, one GPSIMD subtract fills one-hot slots 1-4, the scalar engine
fills slots 0 and 5, and a contiguous-window DVE reduce produces val.
"""

import os
import sys

for _p in ("/opt/trn_rl_repo", "/root/.axon_site", "/root/.axon_site/_ro/trn_rl_repo",
           "/root/.axon_site/_ro/pypackages"):
    if os.path.isdir(_p) and _p not in sys.path:
        sys.path.append(_p)

import numpy as np

from concourse import bass, mybir
from concourse.tile import TileContext
from concourse.bass_utils import run_bass_kernel_spmd

B, D = 2048, 3706
N_CORES = 8
ROWS = B // N_CORES          # 256 rows per core
N_BT = ROWS // 128           # 2 partition tiles per core
EPS = 1e-4
P_B, F_B = 109, 34           # D = 109 * 34 for the boundary-prep layout
# partition-aligned chunks (multiples of 34 = whole t_b partitions); the
# tiny leading chunk warms the PE->compare pipeline during startup
CHUNKS = [34, 306] + [340] * 9 + [306]  # sums to 3706

F32 = mybir.dt.float32
AF = mybir.ActivationFunctionType
ALU = mybir.AluOpType


def _split_excess_waits(nc, max_waits=1):
    """The walrus build in this environment rejects instructions carrying
    more than one semaphore wait. Move excess waits onto preceding
    same-engine NOPs (same-engine program order preserves semantics)."""
    n = 0
    for fn in nc.m.functions:
        for bb in fn.blocks:
            insts = list(bb.instructions)
            if not any(i.sync_info is not None and len(i.sync_info.on_wait) > max_waits
                       for i in insts):
                continue
            new_insts = []
            for ins in insts:
                si = ins.sync_info
                if si is not None and len(si.on_wait) > max_waits:
                    waits = list(si.on_wait)
                    extra, keep = waits[:-max_waits], waits[-max_waits:]
                    idx = 0
                    while extra:
                        chunk, extra = extra[:max_waits], extra[max_waits:]
                        nop = mybir.InstNoOp(
                            name=f"{ins.name}-waitsplit{idx}",
                            sync_info=mybir.SyncInfo(on_wait=chunk, on_update=[]),
                            engine=ins.engine,
                            bass_nofuse=True,
                        )
                        nc.register_instruction(nop, overwrite=True)
                        new_insts.append(nop)
                        idx += 1
                        n += 1
                    ins.sync_info = mybir.SyncInfo(on_wait=keep,
                                                   on_update=list(si.on_update))
                new_insts.append(ins)
            bb.instructions = new_insts
    return n


def _build_nc():
    nc = bass.Bass()
    fake = nc.dram_tensor("fake", [ROWS, D], F32, kind="ExternalInput")
    minb = nc.dram_tensor("minb", [D], F32, kind="ExternalInput")
    ilen = nc.dram_tensor("ilen", [D, 4], F32, kind="ExternalInput")
    dist = nc.dram_tensor("dist", [ROWS, D, 6], F32, kind="ExternalOutput")
    val = nc.dram_tensor("val", [ROWS, D], F32, kind="ExternalOutput")

    with TileContext(nc) as tc:
        with (
            tc.tile_pool(name="setup", bufs=1) as setup_pool,
            tc.tile_pool(name="vin", bufs=2) as vpool,
            tc.tile_pool(name="brow", bufs=3) as browpool,
            tc.tile_pool(name="g", bufs=3) as gpool,
            tc.tile_pool(name="dist", bufs=4) as dpool,
            tc.tile_pool(name="valp", bufs=4) as valpool,
            tc.tile_pool(name="psum", bufs=2, space="PSUM") as ppool,
        ):
            # ---- boundary prep: b[d,k] on a [109, 5, 34] grid (d = p*34+f) ----
            # tiny parameter loads dispatch first so they are not queued
            # behind the megabyte v loads
            t_min = setup_pool.tile([P_B, F_B], F32)
            nc.sync.dma_start(out=t_min[:, :],
                              in_=minb[:].rearrange("(p f) -> p f", p=P_B))
            t_len = setup_pool.tile([P_B, F_B * 4], F32)
            nc.sync.dma_start(
                out=t_len[:, :].rearrange("p (f k) -> p f k", k=4),
                in_=ilen[:, :].rearrange("(p f) k -> p f k", p=P_B))
            # steps = relu(ilen) + eps, layout [109, (f,k)]
            nc.scalar.activation(t_len[:, :], t_len[:, :], AF.Relu)
            nc.vector.tensor_scalar_add(t_len[:, :], t_len[:, :], EPS)
            # t_b interleaved per partition: t_b[p, f*5 + k] = b[p*34+f, k]
            t_b = setup_pool.tile([P_B, 5 * F_B], F32)
            t_b_v = t_b[:, :].rearrange("p (f k) -> p f k", k=5)
            steps_v = t_len[:, :].rearrange("p (f k) -> p f k", k=4)
            nc.vector.tensor_copy(t_b_v[:, :, 0], t_min[:, :])
            for k in range(1, 5):
                nc.vector.tensor_add(
                    out=t_b_v[:, :, k],
                    in0=t_b_v[:, :, k - 1],
                    in1=steps_v[:, :, k - 1])
            # chunks are whole-partition spans of t_b, so each chunk's
            # boundary row is one SBUF->SBUF DMA (no DRAM staging)

            v_t = [vpool.tile([128, D], F32, tag="v", name=f"v{bt}")
                   for bt in range(N_BT)]
            for bt in range(N_BT):
                nc.sync.dma_start(out=v_t[bt][:, :],
                                  in_=fake[bt * 128:(bt + 1) * 128, :])

            # ones weights for the K=1 broadcast matmul
            ones = setup_pool.tile([1, 128], F32)
            nc.vector.memset(ones[:, :], 1.0)

            # ---- main loop ----
            # val accumulates into independent half-tiles so the mid-kernel
            # flush creates no write-after-read coupling with later chunks
            HALF_CI = len(CHUNKS) // 2 - 1
            HALF_D = sum(CHUNKS[:HALF_CI + 1])
            val_t = [[valpool.tile([128, max(HALF_D, D - HALF_D)], F32,
                                   tag="val", name=f"val{bt}h{h}")
                      for h in range(2)] for bt in range(N_BT)]

            def val_slice(bt, lo, hi):
                if hi <= HALF_D:
                    return val_t[bt][0][:, lo:hi]
                return val_t[bt][1][:, lo - HALF_D:hi - HALF_D]

            NMAX = 5 * max(CHUNKS)
            d0 = 0
            val_flushed = 0
            for ci, w in enumerate(CHUNKS):
                n = 5 * w
                np_chunk = w // F_B          # whole t_b partitions in this chunk
                p0 = d0 // F_B
                b_row = browpool.tile([1, NMAX], F32, tag="brow")
                nc.sync.dma_start(
                    out=b_row[0:1, :n].rearrange("q (p x) -> q p x", p=np_chunk),
                    in_=t_b[p0:p0 + np_chunk, :])
                # replicate across partitions: 4 bank-aligned 512-wide matmuls;
                # compares read the result straight from PSUM
                pt = ppool.tile([128, 2048], F32, tag="pt")
                for m0 in range(0, n, 512):
                    mw = min(512, n - m0)
                    nc.tensor.matmul(pt[:, m0:m0 + mw], ones[0:1, :],
                                     b_row[0:1, m0:m0 + mw],
                                     start=True, stop=True)
                B_v = pt[:, :n].rearrange("p (d k) -> p d k", k=5)
                for bt in range(N_BT):
                    vv = v_t[bt][:, d0:d0 + w]
                    g = gpool.tile([128, NMAX], mybir.dt.bfloat16, tag="g")
                    g_v = g[:, :n].rearrange("p (d k) -> p d k", k=5)
                    # one compare: g[p, d, k] = v[p, d] > B[d, k]
                    nc.vector.tensor_tensor(
                        out=g_v, in0=vv.broadcast_to([128, w, 5]),
                        in1=B_v, op=ALU.is_gt)
                    dist_t = dpool.tile([128, max(CHUNKS) * 6], F32, tag="dist")
                    dist_v = dist_t[:, :w * 6].rearrange("p (d r) -> p d r", r=6)
                    # slots 1-4 in one strided subtract on the (idle) pool engine
                    nc.gpsimd.tensor_tensor(
                        out=dist_v[:, :, 1:5],
                        in0=g_v[:, :, 0:4], in1=g_v[:, :, 1:5], op=ALU.subtract)
                    nc.scalar.activation(dist_v[:, :, 0], g_v[:, :, 0], AF.Copy,
                                         bias=1.0, scale=-1.0)
                    nc.scalar.copy(dist_v[:, :, 5], g_v[:, :, 4])
                    nc.vector.tensor_reduce(
                        out=val_slice(bt, d0, d0 + w), in_=g_v,
                        axis=mybir.AxisListType.X, op=ALU.add)
                    nc.sync.dma_start(
                        out=dist[bt * 128:(bt + 1) * 128, d0:d0 + w, :],
                        in_=dist_v[:, :, :])
                d0 += w
                # flush val halves early so the final stores don't serialize
                if ci == HALF_CI or ci == len(CHUNKS) - 1:
                    h = 0 if ci == HALF_CI else 1
                    for bt in range(N_BT):
                        nc.sync.dma_start(
                            out=val[bt * 128:(bt + 1) * 128, val_flushed:d0],
                            in_=val_t[bt][h][:, :d0 - val_flushed])
                    val_flushed = d0

    _split_excess_waits(nc, 1)
    return nc


_NC_CACHE = {}


def _get_nc():
    if "nc" not in _NC_CACHE:
        _NC_CACHE["nc"] = _build_nc()
    return _NC_CACHE["nc"]


def _patch_exact_ties(dist, val, fake, minb, ilen):
    """The device kernel computes dist_r = gt_{r-1} - gt_r, which differs from
    the reference H-product only where fake == b_k EXACTLY (the reference
    yields an all-zero one-hot row and val 0 there; heaviside H(0)=0 on both
    sides of the boundary). Ties are measure-zero; recompute the reference
    formula at just those sites."""
    steps = np.maximum(ilen, 0.0).astype(np.float32) + np.float32(EPS)
    b = np.cumsum(np.concatenate([minb[:, None], steps], axis=1).astype(np.float32),
                  axis=1, dtype=np.float32)  # [D,5]
    rows = np.zeros((0,), np.int64)
    cols = np.zeros((0,), np.int64)
    for k in range(5):
        r, c = np.nonzero(fake == b[None, :, k])
        rows = np.concatenate([rows, r])
        cols = np.concatenate([cols, c])
    if len(rows) == 0:
        return
    for i, d in zip(rows, cols):
        v = fake[i, d]
        gt = (v > b[d]).astype(np.float32)   # [5]
        lt = (v < b[d]).astype(np.float32)
        row = np.empty(6, np.float32)
        row[0] = lt[0]
        for r in range(1, 5):
            row[r] = gt[r - 1] * lt[r]
        row[5] = gt[4]
        dist[i, d, :] = row
        val[i, d] = np.dot(row, np.arange(6.0, dtype=np.float32))


def kernel(fake_tensor, min_boundary_value, interval_lengths):
    fake_tensor = np.ascontiguousarray(np.asarray(fake_tensor, dtype=np.float32))
    minb = np.ascontiguousarray(np.asarray(min_boundary_value, dtype=np.float32))
    ilen = np.ascontiguousarray(np.asarray(interval_lengths, dtype=np.float32))
    assert fake_tensor.shape == (B, D)

    nc = _get_nc()
    in_maps = []
    for c in range(N_CORES):
        in_maps.append({
            "fake": fake_tensor[c * ROWS:(c + 1) * ROWS],
            "minb": minb,
            "ilen": ilen,
        })
    res = run_bass_kernel_spmd(nc, in_maps, core_ids=list(range(N_CORES)))
    dist = np.concatenate([res.results[c]["dist"] for c in range(N_CORES)], axis=0)
    val = np.concatenate([res.results[c]["val"] for c in range(N_CORES)], axis=0)
    _patch_exact_ties(dist, val, fake_tensor, minb, ilen)
    return dist, val


# revision 24
# speedup vs baseline: 1.2246x; 1.1114x over previous
"""Trainium2 Bass kernel for nn_BaseDiscretGenerator (histogram binning).

reference semantics:
    steps = relu(interval_lengths) + 1e-4                    # [D,4]
    b = cumsum([min_boundary, steps])                        # [D,5] strictly increasing
    g_k = (v > b_k)                                          # [B,D,5]
    dist = [1-g0, g0-g1, g1-g2, g2-g3, g3-g4, g4]            # [B,D,6] one-hot bin
    val  = sum_k g_k                                         # [B,D] bin index

Sharded batch-parallel over 8 NeuronCores (256 rows each). Inside each core:
partition dim = batch rows, free dim = feature (D) chunks, all intermediate
tensors in the d-interleaved layout [d*5+k] so each stage is one wide
instruction. Boundaries are computed on-chip, staged to DRAM as an
interleaved flat row, broadcast across the 128 partitions with K=1 PE
matmuls (ones-vector weights, 512-wide bank-aligned), and copied from PSUM
to SBUF by the scalar engine. Per tile: one DVE compare produces all five
g planes, one GPSIMD subtract fills one-hot slots 1-4, the scalar engine
fills slots 0 and 5, and a contiguous-window DVE reduce produces val.
"""

import os
import sys

for _p in ("/opt/trn_rl_repo", "/root/.axon_site", "/root/.axon_site/_ro/trn_rl_repo",
           "/root/.axon_site/_ro/pypackages"):
    if os.path.isdir(_p) and _p not in sys.path:
        sys.path.append(_p)

import numpy as np

from concourse import bass, mybir
from concourse.tile import TileContext
from concourse.bass_utils import run_bass_kernel_spmd

B, D = 2048, 3706
N_CORES = 8
ROWS = B // N_CORES          # 256 rows per core
N_BT = ROWS // 128           # 2 partition tiles per core
EPS = 1e-4
P_B, F_B = 109, 34           # D = 109 * 34 for the boundary-prep layout
# partition-aligned chunks (multiples of 34 = whole t_b partitions); the
# tiny leading chunk warms the PE->compare pipeline during startup
CHUNKS = [34, 306] + [340] * 9 + [306]  # sums to 3706

F32 = mybir.dt.float32
AF = mybir.ActivationFunctionType
ALU = mybir.AluOpType


def _split_excess_waits(nc, max_waits=1):
    """The walrus build in this environment rejects instructions carrying
    more than one semaphore wait. Move excess waits onto preceding
    same-engine NOPs (same-engine program order preserves semantics)."""
    n = 0
    for fn in nc.m.functions:
        for bb in fn.blocks:
            insts = list(bb.instructions)
            if not any(i.sync_info is not None and len(i.sync_info.on_wait) > max_waits
                       for i in insts):
                continue
            new_insts = []
            for ins in insts:
                si = ins.sync_info
                if si is not None and len(si.on_wait) > max_waits:
                    waits = list(si.on_wait)
                    extra, keep = waits[:-max_waits], waits[-max_waits:]
                    idx = 0
                    while extra:
                        chunk, extra = extra[:max_waits], extra[max_waits:]
                        nop = mybir.InstNoOp(
                            name=f"{ins.name}-waitsplit{idx}",
                            sync_info=mybir.SyncInfo(on_wait=chunk, on_update=[]),
                            engine=ins.engine,
                            bass_nofuse=True,
                        )
                        nc.register_instruction(nop, overwrite=True)
                        new_insts.append(nop)
                        idx += 1
                        n += 1
                    ins.sync_info = mybir.SyncInfo(on_wait=keep,
                                                   on_update=list(si.on_update))
                new_insts.append(ins)
            bb.instructions = new_insts
    return n


def _build_nc():
    nc = bass.Bass()
    fake = nc.dram_tensor("fake", [ROWS, D], F32, kind="ExternalInput")
    minb = nc.dram_tensor("minb", [D], F32, kind="ExternalInput")
    ilen = nc.dram_tensor("ilen", [D, 4], F32, kind="ExternalInput")
    dist = nc.dram_tensor("dist", [ROWS, D, 6], F32, kind="ExternalOutput")
    val = nc.dram_tensor("val", [ROWS, D], F32, kind="ExternalOutput")

    with TileContext(nc) as tc:
        with (
            tc.tile_pool(name="setup", bufs=1) as setup_pool,
            tc.tile_pool(name="vin", bufs=2) as vpool,
            tc.tile_pool(name="brow", bufs=6) as browpool,
            tc.tile_pool(name="g", bufs=3) as gpool,
            tc.tile_pool(name="dist", bufs=4) as dpool,
            tc.tile_pool(name="valp", bufs=4) as valpool,
            tc.tile_pool(name="psum", bufs=2, space="PSUM") as ppool,
        ):
            # ---- boundary prep: b[d,k] on a [109, 5, 34] grid (d = p*34+f) ----
            # tiny parameter loads dispatch first so they are not queued
            # behind the megabyte v loads
            t_min = setup_pool.tile([P_B, F_B], F32)
            nc.scalar.dma_start(out=t_min[:, :],
                                in_=minb[:].rearrange("(p f) -> p f", p=P_B))
            t_len = setup_pool.tile([P_B, F_B * 4], F32)
            nc.scalar.dma_start(
                out=t_len[:, :].rearrange("p (f k) -> p f k", k=4),
                in_=ilen[:, :].rearrange("(p f) k -> p f k", p=P_B))
            # steps = relu(ilen) + eps, layout [109, (f,k)]
            nc.scalar.activation(t_len[:, :], t_len[:, :], AF.Relu)
            nc.vector.tensor_scalar_add(t_len[:, :], t_len[:, :], EPS)
            # t_b interleaved per partition: t_b[p, f*5 + k] = b[p*34+f, k]
            t_b = setup_pool.tile([P_B, 5 * F_B], F32)
            t_b_v = t_b[:, :].rearrange("p (f k) -> p f k", k=5)
            steps_v = t_len[:, :].rearrange("p (f k) -> p f k", k=4)
            nc.vector.tensor_copy(t_b_v[:, :, 0], t_min[:, :])
            for k in range(1, 5):
                nc.vector.tensor_add(
                    out=t_b_v[:, :, k],
                    in0=t_b_v[:, :, k - 1],
                    in1=steps_v[:, :, k - 1])
            # chunks are whole-partition spans of t_b, so each chunk's
            # boundary row is one SBUF->SBUF DMA (no DRAM staging)

            v_t = [vpool.tile([128, D], F32, tag="v", name=f"v{bt}")
                   for bt in range(N_BT)]
            for bt in range(N_BT):
                nc.scalar.dma_start(out=v_t[bt][:, :],
                                    in_=fake[bt * 128:(bt + 1) * 128, :])

            # ones weights for the K=1 broadcast matmul
            ones = setup_pool.tile([1, 128], F32)
            nc.vector.memset(ones[:, :], 1.0)

            # ---- main loop ----
            # val accumulates into independent half-tiles so the mid-kernel
            # flush creates no write-after-read coupling with later chunks
            HALF_CI = len(CHUNKS) // 2 - 1
            HALF_D = sum(CHUNKS[:HALF_CI + 1])
            val_t = [[valpool.tile([128, max(HALF_D, D - HALF_D)], F32,
                                   tag="val", name=f"val{bt}h{h}")
                      for h in range(2)] for bt in range(N_BT)]

            def val_slice(bt, lo, hi):
                if hi <= HALF_D:
                    return val_t[bt][0][:, lo:hi]
                return val_t[bt][1][:, lo - HALF_D:hi - HALF_D]

            NMAX = 5 * max(CHUNKS)
            d0 = 0
            val_flushed = 0
            for ci, w in enumerate(CHUNKS):
                n = 5 * w
                np_chunk = w // F_B          # whole t_b partitions in this chunk
                p0 = d0 // F_B
                b_row = browpool.tile([1, NMAX], F32, tag="brow")
                nc.scalar.dma_start(
                    out=b_row[0:1, :n].rearrange("q (p x) -> q p x", p=np_chunk),
                    in_=t_b[p0:p0 + np_chunk, :])
                # replicate across partitions: 4 bank-aligned 512-wide matmuls;
                # compares read the result straight from PSUM
                pt = ppool.tile([128, 2048], F32, tag="pt")
                for m0 in range(0, n, 512):
                    mw = min(512, n - m0)
                    nc.tensor.matmul(pt[:, m0:m0 + mw], ones[0:1, :],
                                     b_row[0:1, m0:m0 + mw],
                                     start=True, stop=True)
                B_v = pt[:, :n].rearrange("p (d k) -> p d k", k=5)
                for bt in range(N_BT):
                    vv = v_t[bt][:, d0:d0 + w]
                    g = gpool.tile([128, NMAX], mybir.dt.bfloat16, tag="g")
                    g_v = g[:, :n].rearrange("p (d k) -> p d k", k=5)
                    # one compare: g[p, d, k] = v[p, d] > B[d, k]
                    nc.vector.tensor_tensor(
                        out=g_v, in0=vv.broadcast_to([128, w, 5]),
                        in1=B_v, op=ALU.is_gt)
                    dist_t = dpool.tile([128, max(CHUNKS) * 6], F32, tag="dist")
                    dist_v = dist_t[:, :w * 6].rearrange("p (d r) -> p d r", r=6)
                    # slots 1-4 in one strided subtract on the (idle) pool engine
                    nc.gpsimd.tensor_tensor(
                        out=dist_v[:, :, 1:5],
                        in0=g_v[:, :, 0:4], in1=g_v[:, :, 1:5], op=ALU.subtract)
                    nc.scalar.activation(dist_v[:, :, 0], g_v[:, :, 0], AF.Copy,
                                         bias=1.0, scale=-1.0)
                    nc.scalar.copy(dist_v[:, :, 5], g_v[:, :, 4])
                    nc.vector.tensor_reduce(
                        out=val_slice(bt, d0, d0 + w), in_=g_v,
                        axis=mybir.AxisListType.X, op=ALU.add)
                    nc.sync.dma_start(
                        out=dist[bt * 128:(bt + 1) * 128, d0:d0 + w, :],
                        in_=dist_v[:, :, :])
                d0 += w
                # flush val halves early so the final stores don't serialize
                if ci == HALF_CI or ci == len(CHUNKS) - 1:
                    h = 0 if ci == HALF_CI else 1
                    for bt in range(N_BT):
                        nc.sync.dma_start(
                            out=val[bt * 128:(bt + 1) * 128, val_flushed:d0],
                            in_=val_t[bt][h][:, :d0 - val_flushed])
                    val_flushed = d0

    _split_excess_waits(nc, 1)
    return nc


_NC_CACHE = {}


def _get_nc():
    if "nc" not in _NC_CACHE:
        _NC_CACHE["nc"] = _build_nc()
    return _NC_CACHE["nc"]


def _patch_exact_ties(dist, val, fake, minb, ilen):
    """The device kernel computes dist_r = gt_{r-1} - gt_r, which differs from
    the reference H-product only where fake == b_k EXACTLY (the reference
    yields an all-zero one-hot row and val 0 there; heaviside H(0)=0 on both
    sides of the boundary). Ties are measure-zero; recompute the reference
    formula at just those sites."""
    steps = np.maximum(ilen, 0.0).astype(np.float32) + np.float32(EPS)
    b = np.cumsum(np.concatenate([minb[:, None], steps], axis=1).astype(np.float32),
                  axis=1, dtype=np.float32)  # [D,5]
    rows = np.zeros((0,), np.int64)
    cols = np.zeros((0,), np.int64)
    for k in range(5):
        r, c = np.nonzero(fake == b[None, :, k])
        rows = np.concatenate([rows, r])
        cols = np.concatenate([cols, c])
    if len(rows) == 0:
        return
    for i, d in zip(rows, cols):
        v = fake[i, d]
        gt = (v > b[d]).astype(np.float32)   # [5]
        lt = (v < b[d]).astype(np.float32)
        row = np.empty(6, np.float32)
        row[0] = lt[0]
        for r in range(1, 5):
            row[r] = gt[r - 1] * lt[r]
        row[5] = gt[4]
        dist[i, d, :] = row
        val[i, d] = np.dot(row, np.arange(6.0, dtype=np.float32))


def kernel(fake_tensor, min_boundary_value, interval_lengths):
    fake_tensor = np.ascontiguousarray(np.asarray(fake_tensor, dtype=np.float32))
    minb = np.ascontiguousarray(np.asarray(min_boundary_value, dtype=np.float32))
    ilen = np.ascontiguousarray(np.asarray(interval_lengths, dtype=np.float32))
    assert fake_tensor.shape == (B, D)

    nc = _get_nc()
    in_maps = []
    for c in range(N_CORES):
        in_maps.append({
            "fake": fake_tensor[c * ROWS:(c + 1) * ROWS],
            "minb": minb,
            "ilen": ilen,
        })
    res = run_bass_kernel_spmd(nc, in_maps, core_ids=list(range(N_CORES)))
    dist = np.concatenate([res.results[c]["dist"] for c in range(N_CORES)], axis=0)
    val = np.concatenate([res.results[c]["val"] for c in range(N_CORES)], axis=0)
    _patch_exact_ties(dist, val, fake_tensor, minb, ilen)
    return dist, val


# revision 25
# speedup vs baseline: 1.2310x; 1.0053x over previous
"""Trainium2 Bass kernel for nn_BaseDiscretGenerator (histogram binning).

reference semantics:
    steps = relu(interval_lengths) + 1e-4                    # [D,4]
    b = cumsum([min_boundary, steps])                        # [D,5] strictly increasing
    g_k = (v > b_k)                                          # [B,D,5]
    dist = [1-g0, g0-g1, g1-g2, g2-g3, g3-g4, g4]            # [B,D,6] one-hot bin
    val  = sum_k g_k                                         # [B,D] bin index

Sharded batch-parallel over 8 NeuronCores (256 rows each). Inside each core:
partition dim = batch rows, free dim = feature (D) chunks, all intermediate
tensors in the d-interleaved layout [d*5+k] so each stage is one wide
instruction. Boundaries are computed on-chip, staged to DRAM as an
interleaved flat row, broadcast across the 128 partitions with K=1 PE
matmuls (ones-vector weights, 512-wide bank-aligned), and copied from PSUM
to SBUF by the scalar engine. Per tile: one DVE compare produces all five
g planes, one GPSIMD subtract fills one-hot slots 1-4, the scalar engine
fills slots 0 and 5, and a contiguous-window DVE reduce produces val.
"""

import os
import sys

for _p in ("/opt/trn_rl_repo", "/root/.axon_site", "/root/.axon_site/_ro/trn_rl_repo",
           "/root/.axon_site/_ro/pypackages"):
    if os.path.isdir(_p) and _p not in sys.path:
        sys.path.append(_p)

import numpy as np

from concourse import bass, mybir
from concourse.tile import TileContext
from concourse.bass_utils import run_bass_kernel_spmd

B, D = 2048, 3706
N_CORES = 8
ROWS = B // N_CORES          # 256 rows per core
N_BT = ROWS // 128           # 2 partition tiles per core
EPS = 1e-4
P_B, F_B = 109, 34           # D = 109 * 34 for the boundary-prep layout
# partition-aligned chunks (multiples of 34 = whole t_b partitions); sizes
# ramp up so the PE->compare pipeline fills during startup and the final
# chunk is tiny so almost no store traffic remains after compute ends
CHUNKS = [34, 102, 204] + [340] * 9 + [272, 34]  # sums to 3706

F32 = mybir.dt.float32
AF = mybir.ActivationFunctionType
ALU = mybir.AluOpType


def _split_excess_waits(nc, max_waits=1):
    """The walrus build in this environment rejects instructions carrying
    more than one semaphore wait. Move excess waits onto preceding
    same-engine NOPs (same-engine program order preserves semantics)."""
    n = 0
    for fn in nc.m.functions:
        for bb in fn.blocks:
            insts = list(bb.instructions)
            if not any(i.sync_info is not None and len(i.sync_info.on_wait) > max_waits
                       for i in insts):
                continue
            new_insts = []
            for ins in insts:
                si = ins.sync_info
                if si is not None and len(si.on_wait) > max_waits:
                    waits = list(si.on_wait)
                    extra, keep = waits[:-max_waits], waits[-max_waits:]
                    idx = 0
                    while extra:
                        chunk, extra = extra[:max_waits], extra[max_waits:]
                        nop = mybir.InstNoOp(
                            name=f"{ins.name}-waitsplit{idx}",
                            sync_info=mybir.SyncInfo(on_wait=chunk, on_update=[]),
                            engine=ins.engine,
                            bass_nofuse=True,
                        )
                        nc.register_instruction(nop, overwrite=True)
                        new_insts.append(nop)
                        idx += 1
                        n += 1
                    ins.sync_info = mybir.SyncInfo(on_wait=keep,
                                                   on_update=list(si.on_update))
                new_insts.append(ins)
            bb.instructions = new_insts
    return n


def _build_nc():
    nc = bass.Bass()
    fake = nc.dram_tensor("fake", [ROWS, D], F32, kind="ExternalInput")
    minb = nc.dram_tensor("minb", [D], F32, kind="ExternalInput")
    ilen = nc.dram_tensor("ilen", [D, 4], F32, kind="ExternalInput")
    dist = nc.dram_tensor("dist", [ROWS, D, 6], F32, kind="ExternalOutput")
    val = nc.dram_tensor("val", [ROWS, D], F32, kind="ExternalOutput")

    with TileContext(nc) as tc:
        with (
            tc.tile_pool(name="setup", bufs=1) as setup_pool,
            tc.tile_pool(name="vin", bufs=2) as vpool,
            tc.tile_pool(name="brow", bufs=6) as browpool,
            tc.tile_pool(name="g", bufs=3) as gpool,
            tc.tile_pool(name="dist", bufs=4) as dpool,
            tc.tile_pool(name="valp", bufs=6) as valpool,
            tc.tile_pool(name="psum", bufs=2, space="PSUM") as ppool,
        ):
            # ---- boundary prep: b[d,k] on a [109, 5, 34] grid (d = p*34+f) ----
            # tiny parameter loads dispatch first so they are not queued
            # behind the megabyte v loads
            t_min = setup_pool.tile([P_B, F_B], F32)
            nc.scalar.dma_start(out=t_min[:, :],
                                in_=minb[:].rearrange("(p f) -> p f", p=P_B))
            t_len = setup_pool.tile([P_B, F_B * 4], F32)
            nc.scalar.dma_start(
                out=t_len[:, :].rearrange("p (f k) -> p f k", k=4),
                in_=ilen[:, :].rearrange("(p f) k -> p f k", p=P_B))
            # steps = relu(ilen) + eps, layout [109, (f,k)]
            nc.scalar.activation(t_len[:, :], t_len[:, :], AF.Relu)
            nc.vector.tensor_scalar_add(t_len[:, :], t_len[:, :], EPS)
            # t_b interleaved per partition: t_b[p, f*5 + k] = b[p*34+f, k]
            t_b = setup_pool.tile([P_B, 5 * F_B], F32)
            t_b_v = t_b[:, :].rearrange("p (f k) -> p f k", k=5)
            steps_v = t_len[:, :].rearrange("p (f k) -> p f k", k=4)
            nc.vector.tensor_copy(t_b_v[:, :, 0], t_min[:, :])
            for k in range(1, 5):
                nc.vector.tensor_add(
                    out=t_b_v[:, :, k],
                    in0=t_b_v[:, :, k - 1],
                    in1=steps_v[:, :, k - 1])
            # chunks are whole-partition spans of t_b, so each chunk's
            # boundary row is one SBUF->SBUF DMA (no DRAM staging)

            v_t = [vpool.tile([128, D], F32, tag="v", name=f"v{bt}")
                   for bt in range(N_BT)]
            for bt in range(N_BT):
                nc.scalar.dma_start(out=v_t[bt][:, :],
                                    in_=fake[bt * 128:(bt + 1) * 128, :])

            # ones weights for the K=1 broadcast matmul
            ones = setup_pool.tile([1, 128], F32)
            nc.vector.memset(ones[:, :], 1.0)

            # ---- main loop ----
            # val accumulates into independent piece-tiles, flushed as each
            # piece completes (no write-after-read coupling, tiny tail)
            nch = len(CHUNKS)
            cum = [0]
            for w in CHUNKS:
                cum.append(cum[-1] + w)
            half_ci = next(i for i in range(nch) if cum[i + 1] >= D // 2)
            FLUSH_CIS = [half_ci, nch - 2, nch - 1]
            PIECE_LO = [0, cum[half_ci + 1], cum[nch - 1]]
            PIECE_HI = [cum[half_ci + 1], cum[nch - 1], D]
            pmax = max(hi - lo for lo, hi in zip(PIECE_LO, PIECE_HI))
            val_t = [[valpool.tile([128, pmax], F32,
                                   tag="val", name=f"val{bt}h{h}")
                      for h in range(3)] for bt in range(N_BT)]

            def val_slice(bt, lo, hi):
                for h in range(3):
                    if lo >= PIECE_LO[h] and hi <= PIECE_HI[h]:
                        return val_t[bt][h][:, lo - PIECE_LO[h]:hi - PIECE_LO[h]]
                raise AssertionError((lo, hi))

            NMAX = 5 * max(CHUNKS)
            d0 = 0
            val_flushed = 0
            for ci, w in enumerate(CHUNKS):
                n = 5 * w
                np_chunk = w // F_B          # whole t_b partitions in this chunk
                p0 = d0 // F_B
                b_row = browpool.tile([1, NMAX], F32, tag="brow")
                nc.scalar.dma_start(
                    out=b_row[0:1, :n].rearrange("q (p x) -> q p x", p=np_chunk),
                    in_=t_b[p0:p0 + np_chunk, :])
                # replicate across partitions: 4 bank-aligned 512-wide matmuls;
                # compares read the result straight from PSUM
                pt = ppool.tile([128, 2048], F32, tag="pt")
                for m0 in range(0, n, 512):
                    mw = min(512, n - m0)
                    nc.tensor.matmul(pt[:, m0:m0 + mw], ones[0:1, :],
                                     b_row[0:1, m0:m0 + mw],
                                     start=True, stop=True)
                B_v = pt[:, :n].rearrange("p (d k) -> p d k", k=5)
                for bt in range(N_BT):
                    vv = v_t[bt][:, d0:d0 + w]
                    g = gpool.tile([128, NMAX], mybir.dt.bfloat16, tag="g")
                    g_v = g[:, :n].rearrange("p (d k) -> p d k", k=5)
                    # one compare: g[p, d, k] = v[p, d] > B[d, k]
                    nc.vector.tensor_tensor(
                        out=g_v, in0=vv.broadcast_to([128, w, 5]),
                        in1=B_v, op=ALU.is_gt)
                    dist_t = dpool.tile([128, max(CHUNKS) * 6], F32, tag="dist")
                    dist_v = dist_t[:, :w * 6].rearrange("p (d r) -> p d r", r=6)
                    # slots 1-4 in one strided subtract on the (idle) pool engine
                    nc.gpsimd.tensor_tensor(
                        out=dist_v[:, :, 1:5],
                        in0=g_v[:, :, 0:4], in1=g_v[:, :, 1:5], op=ALU.subtract)
                    nc.scalar.activation(dist_v[:, :, 0], g_v[:, :, 0], AF.Copy,
                                         bias=1.0, scale=-1.0)
                    nc.scalar.copy(dist_v[:, :, 5], g_v[:, :, 4])
                    nc.vector.tensor_reduce(
                        out=val_slice(bt, d0, d0 + w), in_=g_v,
                        axis=mybir.AxisListType.X, op=ALU.add)
                    nc.sync.dma_start(
                        out=dist[bt * 128:(bt + 1) * 128, d0:d0 + w, :],
                        in_=dist_v[:, :, :])
                d0 += w
                # flush each completed val piece so final stores don't serialize
                if ci in FLUSH_CIS:
                    h = FLUSH_CIS.index(ci)
                    for bt in range(N_BT):
                        nc.sync.dma_start(
                            out=val[bt * 128:(bt + 1) * 128, val_flushed:d0],
                            in_=val_t[bt][h][:, :d0 - val_flushed])
                    val_flushed = d0

    _split_excess_waits(nc, 1)
    return nc


_NC_CACHE = {}


def _get_nc():
    if "nc" not in _NC_CACHE:
        _NC_CACHE["nc"] = _build_nc()
    return _NC_CACHE["nc"]


def _patch_exact_ties(dist, val, fake, minb, ilen):
    """The device kernel computes dist_r = gt_{r-1} - gt_r, which differs from
    the reference H-product only where fake == b_k EXACTLY (the reference
    yields an all-zero one-hot row and val 0 there; heaviside H(0)=0 on both
    sides of the boundary). Ties are measure-zero; recompute the reference
    formula at just those sites."""
    steps = np.maximum(ilen, 0.0).astype(np.float32) + np.float32(EPS)
    b = np.cumsum(np.concatenate([minb[:, None], steps], axis=1).astype(np.float32),
                  axis=1, dtype=np.float32)  # [D,5]
    rows = np.zeros((0,), np.int64)
    cols = np.zeros((0,), np.int64)
    for k in range(5):
        r, c = np.nonzero(fake == b[None, :, k])
        rows = np.concatenate([rows, r])
        cols = np.concatenate([cols, c])
    if len(rows) == 0:
        return
    for i, d in zip(rows, cols):
        v = fake[i, d]
        gt = (v > b[d]).astype(np.float32)   # [5]
        lt = (v < b[d]).astype(np.float32)
        row = np.empty(6, np.float32)
        row[0] = lt[0]
        for r in range(1, 5):
            row[r] = gt[r - 1] * lt[r]
        row[5] = gt[4]
        dist[i, d, :] = row
        val[i, d] = np.dot(row, np.arange(6.0, dtype=np.float32))


def kernel(fake_tensor, min_boundary_value, interval_lengths):
    fake_tensor = np.ascontiguousarray(np.asarray(fake_tensor, dtype=np.float32))
    minb = np.ascontiguousarray(np.asarray(min_boundary_value, dtype=np.float32))
    ilen = np.ascontiguousarray(np.asarray(interval_lengths, dtype=np.float32))
    assert fake_tensor.shape == (B, D)

    nc = _get_nc()
    in_maps = []
    for c in range(N_CORES):
        in_maps.append({
            "fake": fake_tensor[c * ROWS:(c + 1) * ROWS],
            "minb": minb,
            "ilen": ilen,
        })
    res = run_bass_kernel_spmd(nc, in_maps, core_ids=list(range(N_CORES)))
    dist = np.concatenate([res.results[c]["dist"] for c in range(N_CORES)], axis=0)
    val = np.concatenate([res.results[c]["val"] for c in range(N_CORES)], axis=0)
    _patch_exact_ties(dist, val, fake_tensor, minb, ilen)
    return dist, val
